# revision 14
# baseline (speedup 1.0000x reference)
"""Bidirectional attention block (B=4, S=2048, H=1024, NH=16, HD=64, FF=4096)
on 8 TRN2 NeuronCores.

Sharding: data-parallel over (batch, sequence-half). Core c handles batch
b = c//2 and query rows q = (c%2)*1024 .. +1024. Each core recomputes K/V for
its batch's full sequence (no cross-core collectives). The per-core input
sequence is rolled so the core's query tokens are always rows 0..1023.

v2: fp8 (e4m3) everywhere except the MLP matmuls and LN/softmax statistics.
  - QKV / PV / out-proj matmuls run in fp8 DoubleRow perf mode (2 k-subtiles
    of 128 per instruction, 0.5 cycles/row -> 4x bf16 on contraction-bound
    matmuls). Scores matmuls are fp8 (same rate as bf16; they are
    output-row-bound at contraction 64).
  - Scale juggling keeps every fp8 tensor in e4m3's sweet range: Wq/Wk x8,
    Wv x16, Wo x8. Scores carry x64, folded into the softmax exp scale.
    Attention output carries x16, folded out at the out-proj PSUM readout
    together with Wo's x8 (/128).
  - Softmax: exp(s - 2) on ACT straight out of PSUM into fp8; the constant
    shift cancels in normalization and keeps exp() clear of e4m3's top end.
    Denominators come free via a ones-column in V (PV accumulates sum(P)).
  - LN1/LN2 transposes ride the DMA xbar (dma_start_transpose, one issue per
    [128,1024] tile) instead of PE+DVE.
  - MLP stays bf16 (fp8 there fails the 2e-2 gate; measured 2.3e-2). MLP1
    PSUMs are drained pre-gelu by DVE; gelu runs on ACT in per-chunk batches
    to amortize the Exp<->Gelu activation-table reloads.
  - Attention (ACT-bound: 256us of exp) is interleaved at emission level with
    the MLP (PE-bound) of earlier 256-token chunks so both engines stay busy.
"""

from contextlib import ExitStack

import numpy as np
import ml_dtypes

import concourse.bass as bass
import concourse.tile as tile
from concourse import bacc, mybir
from concourse.bass_utils import run_bass_kernel_spmd

F32 = mybir.dt.float32
BF16 = mybir.dt.bfloat16
FP8 = mybir.dt.float8e4
DR = mybir.MatmulPerfMode.DoubleRow

B, S, H = 4, 2048, 1024
NH, HD = 16, 64
HD1 = HD + 1
HD2 = 2 * HD   # PV DR lhsT padded to 128 cols: walrus only accepts 32/64/128-wide ldweights subtiles
FF = 4 * H
EPS = 1e-5
P = 128
QT_N = S // 2          # query tokens per core = 1024
HT = H // P            # 8 h-tiles
NTOK = S // P          # 16 token tiles
NKP = NTOK // 2        # 8 key-tile pairs
FT = FF // P           # 32 f-tiles
NB = 4                 # attention blocks
QB = QT_N // NB        # 256 queries per block
SCALE = 1.0 / np.sqrt(HD)
EXP_SCALE = float(SCALE / 64.0)   # Wq,Wk carry x8 each
EXP_SHIFT = -2.0
OP_DESCALE = 1.0 / 128.0          # attn x16 * Wo x8

_CACHED = {}


def _ln_tile(nc, stat, x_t, out_bf, apply_on_act=True):
    """LayerNorm (no affine) of fp32 [128, H] tile -> bf16 tile.

    Statistics stay entirely on DVE (rstd = (var+eps)^-0.5 via the pow ALU
    op) so no ACT table other than Identity is ever needed -- the softmax Exp
    table stays loaded across interleaved LNs. The elementwise apply goes to
    ACT (phase 1, where ACT has slack) or DVE (attention phase, ACT-bound).
    """
    stats = stat.tile([P, 2, nc.vector.BN_STATS_DIM], F32, name="bn_stats", tag="bn_stats")
    xg = x_t.rearrange("p (a b) -> p a b", a=2)
    nc.vector.bn_stats(out=stats[:, 0, :], in_=xg[:, 0, :])
    nc.vector.bn_stats(out=stats[:, 1, :], in_=xg[:, 1, :])
    mv = stat.tile([P, nc.vector.BN_AGGR_DIM], F32, name="bn_mv", tag="bn_mv")
    nc.vector.bn_aggr(out=mv, in_=stats)
    # rstd = (var+eps)^-0.5 on DVE only: quake bit-trick + one Newton step
    # (rel err ~2e-3, irrelevant at this kernel's accuracy budget). Keeps ACT
    # free of Sqrt so the softmax Exp table never reloads mid-attention.
    I32 = mybir.dt.int32
    vp = stat.tile([P, 1], F32, name="bn_vp", tag="bn_vp")
    nc.vector.tensor_scalar(out=vp, in0=mv[:, 1:2], scalar1=EPS, scalar2=None,
                            op0=mybir.AluOpType.add)
    y0i = stat.tile([P, 1], I32, name="bn_y0i", tag="bn_y0i")
    nc.vector.tensor_scalar(out=y0i, in0=vp.bitcast(I32), scalar1=1, scalar2=None,
                            op0=mybir.AluOpType.logical_shift_right)
    nc.vector.tensor_scalar(out=y0i, in0=y0i, scalar1=-1, scalar2=0x5F3759DF,
                            op0=mybir.AluOpType.mult, op1=mybir.AluOpType.add)
    y0 = y0i.bitcast(F32)
    a = stat.tile([P, 1], F32, name="bn_a", tag="bn_a")
    nc.vector.tensor_tensor(out=a, in0=y0, in1=y0, op=mybir.AluOpType.mult)
    nc.vector.tensor_tensor(out=a, in0=a, in1=vp, op=mybir.AluOpType.mult)
    nc.vector.tensor_scalar(out=a, in0=a, scalar1=-0.5, scalar2=1.5,
                            op0=mybir.AluOpType.mult, op1=mybir.AluOpType.add)
    rstd = stat.tile([P, 1], F32, name="bn_rstd", tag="bn_rstd")
    nc.vector.tensor_tensor(out=rstd, in0=y0, in1=a, op=mybir.AluOpType.mult)
    if apply_on_act:
        negmr = stat.tile([P, 1], F32, name="bn_negmr", tag="bn_negmr")
        nc.vector.tensor_scalar(out=negmr, in0=mv[:, 0:1], scalar1=rstd, scalar2=-1.0,
                                op0=mybir.AluOpType.mult, op1=mybir.AluOpType.mult)
        nc.scalar.activation(out=out_bf, in_=x_t,
                             func=mybir.ActivationFunctionType.Identity,
                             bias=negmr, scale=rstd)
    else:
        negm = stat.tile([P, 1], F32, name="bn_negm", tag="bn_negm")
        nc.vector.tensor_scalar(out=negm, in0=mv[:, 0:1], scalar1=-1.0, scalar2=None,
                                op0=mybir.AluOpType.mult)
        nc.vector.tensor_scalar(out=out_bf, in0=x_t, scalar1=negm, scalar2=rstd,
                                op0=mybir.AluOpType.add, op1=mybir.AluOpType.mult)


def build_core_kernel():
    """One SPMD program; every core runs the same code on its own shard."""
    nc = bacc.Bacc(None, target_bir_lowering=False)

    xin = nc.declare_dram_parameter("xin", [S, H], F32, isOutput=False)
    wq8 = nc.declare_dram_parameter("wq8", [H, H], FP8, isOutput=False)
    wk8 = nc.declare_dram_parameter("wk8", [H, H], FP8, isOutput=False)
    wv8 = nc.declare_dram_parameter("wv8", [H, H], FP8, isOutput=False)
    wo8 = nc.declare_dram_parameter("wo8", [H, H], FP8, isOutput=False)
    wm1T = nc.declare_dram_parameter("wm1T", [H, FF], BF16, isOutput=False)
    wm2T = nc.declare_dram_parameter("wm2T", [FF, H], BF16, isOutput=False)
    bqd = nc.declare_dram_parameter("bqd", [HT, P], F32, isOutput=False)
    bkd = nc.declare_dram_parameter("bkd", [HT, P], F32, isOutput=False)
    bvv = nc.declare_dram_parameter("bvv", [1, H], BF16, isOutput=False)
    bov = nc.declare_dram_parameter("bov", [1, H], BF16, isOutput=False)
    bm1d = nc.declare_dram_parameter("bm1d", [FT, P], F32, isOutput=False)
    bm2v = nc.declare_dram_parameter("bm2v", [1, H], BF16, isOutput=False)
    out = nc.declare_dram_parameter("out", [QT_N, H], F32, isOutput=True)

    def dram_bcast(ap_row, cols):
        return bass.AP(tensor=ap_row.tensor, offset=ap_row.offset,
                       ap=[[0, P], [1, cols]])

    with tile.TileContext(nc) as tc, ExitStack() as es:
        const = es.enter_context(tc.tile_pool(name="const", bufs=1))
        stat = es.enter_context(tc.tile_pool(name="stat", bufs=8))
        xload = es.enter_context(tc.tile_pool(name="xload", bufs=3))
        nbp = es.enter_context(tc.tile_pool(name="nbp", bufs=2))
        ntbp = es.enter_context(tc.tile_pool(name="ntbp", bufs=2))
        w1p = es.enter_context(tc.tile_pool(name="w1p", bufs=2))
        w2p = es.enter_context(tc.tile_pool(name="w2p", bufs=4))
        ptile = es.enter_context(tc.tile_pool(name="ptile", bufs=2))
        rtile = es.enter_context(tc.tile_pool(name="rtile", bufs=2))
        oload = es.enter_context(tc.tile_pool(name="oload", bufs=2))
        big = es.enter_context(tc.tile_pool(name="big", bufs=1))
        pp = es.enter_context(tc.tile_pool(name="pp", bufs=1, space="PSUM"))
        dram = es.enter_context(tc.tile_pool(name="dram", bufs=1, space="DRAM"))

        x1_dram = dram.tile([QT_N, H], F32, name="x1_dram", tag="x1_dram")

        # ---- constants ----
        bo_bc = const.tile([P, H], BF16, name="bo_bc", tag="bo_bc")
        nc.gpsimd.dma_start(out=bo_bc, in_=dram_bcast(bov[0:1, :], H))
        bv_bc = const.tile([P, H], BF16, name="bv_bc", tag="bv_bc")
        nc.gpsimd.dma_start(out=bv_bc, in_=dram_bcast(bvv[0:1, :], H))
        bm2_bc = const.tile([P, H], BF16, name="bm2_bc", tag="bm2_bc")
        nc.gpsimd.dma_start(out=bm2_bc, in_=dram_bcast(bm2v[0:1, :], H))
        bqd_t = const.tile([P, HT], F32, name="bqd_t", tag="bqd_t")
        nc.gpsimd.dma_start(out=bqd_t, in_=bqd[:, :].rearrange("a p -> p a"))
        bkd_t = const.tile([P, HT], F32, name="bkd_t", tag="bkd_t")
        nc.gpsimd.dma_start(out=bkd_t, in_=bkd[:, :].rearrange("a p -> p a"))
        bm1d_t = const.tile([P, FT], F32, name="bm1d_t", tag="bm1d_t")
        nc.gpsimd.dma_start(out=bm1d_t, in_=bm1d[:, :].rearrange("a p -> p a"))
        expb = const.tile([P, 1], F32, name="expb", tag="expb")
        nc.vector.memset(expb, EXP_SHIFT)
        zerob = const.tile([P, 1], F32, name="zerob", tag="zerob")
        nc.vector.memset(zerob, 0.0)

        # ---- whole fp8 weights resident ----
        # wq/wk die at phase-1 end; n2T is born at outproj(0): share one slot.
        wqk_a = big.tile([P, 2, HT, H], FP8, name="wqk_a", tag="wqk_n2T")
        wq_a, wk_a = wqk_a[:, 0], wqk_a[:, 1]
        nc.gpsimd.dma_start(out=wq_a, in_=wq8[:, :].rearrange("(a p) d -> p a d", p=P))
        nc.gpsimd.dma_start(out=wk_a, in_=wk8[:, :].rearrange("(a p) d -> p a d", p=P))
        wv_a = big.tile([P, HT, H], FP8, name="wv_a", tag="wv_a")
        nc.sync.dma_start(out=wv_a, in_=wv8[:, :].rearrange("(a p) d -> p a d", p=P))
        wo_a = big.tile([P, HT, H], FP8, name="wo_a", tag="wo_a")
        nc.sync.dma_start(out=wo_a, in_=wo8[:, :].rearrange("(a p) d -> p a d", p=P))

        # ---- long-lived activations ----
        # NT dies at phase-1 end; h1 chunks 0/2 reuse its slot.
        NT = big.tile([P, HT, S], FP8, name="NT", tag="NT_h1")
        KT = [big.tile([P, S], FP8, name=f"KT{i}", tag=f"KT{i}") for i in range(HT)]
        QT = [big.tile([P, QT_N], FP8, name=f"QT{i}", tag=f"QT{i}") for i in range(HT)]
        VS = [big.tile([P, NH, 2, HD2], FP8, name=f"VS{i}", tag=f"VS{i}") for i in range(NKP)]
        for kp in range(NKP):
            nc.gpsimd.memset(VS[kp][:, :, :, HD:HD1], 1.0)
            nc.gpsimd.memset(VS[kp][:, :, :, HD1:HD2], 0.0)   # rows 65.. accumulate zeros

        n2T = big.tile([P, HT, QT_N], BF16, name="n2T", tag="wqk_n2T")
        h1 = [big.tile([P, FT, QB], BF16, name=f"h1_{i}",
                       tag=("NT_h1" if i % 2 == 0 else "h1b"))
              for i in range(NB)]

        _rot = [0]

        def ps512(tags=("PB0", "PB1", "PB2", "PB3")):
            _rot[0] += 1
            return pp.tile([P, 512], F32, name="ps512", tag=tags[_rot[0] % len(tags)])

        # ================= phase 1: LN1 + transpose + QKV =================
        def phase1_tt(tt):
            x_t = xload.tile([P, H], F32, name="xbuf", tag="xbuf", bufs=2)
            nc.sync.dma_start(out=x_t, in_=xin[tt * P:(tt + 1) * P, :])
            nb = nbp.tile([P, H], BF16, name="nb", tag="nb")
            _ln_tile(nc, stat, x_t, nb)
            ntb = ntbp.tile([P, HT, P], BF16, name="ntb", tag="ntb")
            nc.sync.dma_start_transpose(out=ntb, in_=nb)
            nc.gpsimd.tensor_copy(out=NT[:, :, tt * P:(tt + 1) * P], in_=ntb)
            # V projection of this token tile (tokens-major out)
            for c in range(2):
                ps = ps512()
                for t2 in range(4):
                    nc.tensor.matmul(ps,
                                     lhsT=NT[:, 2 * t2:2 * t2 + 2, tt * P:(tt + 1) * P],
                                     rhs=wv_a[:, 2 * t2:2 * t2 + 2, c * 512:(c + 1) * 512],
                                     start=(t2 == 0), stop=(t2 == 3), perf_mode=DR)
                dst = VS[tt // 2][:, c * 8:(c + 1) * 8, tt % 2, 0:HD]
                src = ps.rearrange("p (a b) -> p a b", a=8)
                bvs = bv_bc[:, c * 512:(c + 1) * 512].rearrange("p (a b) -> p a b", a=8)
                nc.vector.tensor_tensor(out=dst, in0=src, in1=bvs, op=mybir.AluOpType.add)

        def phase1_kq(qb):
            cs = slice(qb * 512, (qb + 1) * 512)
            for dt in range(HT):
                ps = ps512()
                for t2 in range(4):
                    nc.tensor.matmul(ps,
                                     lhsT=wk_a[:, 2 * t2:2 * t2 + 2, dt * P:(dt + 1) * P],
                                     rhs=NT[:, 2 * t2:2 * t2 + 2, cs],
                                     start=(t2 == 0), stop=(t2 == 3), perf_mode=DR)
                if dt % 2 == 0:
                    nc.vector.tensor_scalar(out=KT[dt][:, cs], in0=ps,
                                            scalar1=bkd_t[:, dt:dt + 1], scalar2=None,
                                            op0=mybir.AluOpType.add)
                else:
                    nc.scalar.activation(out=KT[dt][:, cs], in_=ps,
                                         func=mybir.ActivationFunctionType.Identity,
                                         bias=bkd_t[:, dt:dt + 1], scale=1.0)
            if qb < 2:
                for dt in range(HT):
                    ps = ps512()
                    for t2 in range(4):
                        nc.tensor.matmul(ps,
                                         lhsT=wq_a[:, 2 * t2:2 * t2 + 2, dt * P:(dt + 1) * P],
                                         rhs=NT[:, 2 * t2:2 * t2 + 2, cs],
                                         start=(t2 == 0), stop=(t2 == 3), perf_mode=DR)
                    nc.scalar.activation(out=QT[dt][:, cs], in_=ps,
                                         func=mybir.ActivationFunctionType.Identity,
                                         bias=bqd_t[:, dt:dt + 1], scale=1.0)

        # tt 8..15 and K(qb2/qb3) ride inside attn(0) as pre-step closures:
        # slot s runs before (hp0, grp s)'s scores, so K(qb_g) lands in slot g
        # and its NT dependencies in slots <= g. Blocks b0/b1 only read
        # Q columns 0:512 (= Q(qb0)/Q(qb1), emitted inline).
        for tt in range(8):
            phase1_tt(tt)
            if tt % 4 == 3:
                phase1_kq(tt // 4)

        # ================= attention / MLP interleaved ====================
        attn_blk = {}
        LOG2E = float(np.log2(np.e))
        SCH_A = float(EXP_SCALE * LOG2E * 8.0)
        SCH_B = float((7.0 + EXP_SHIFT * LOG2E) * 8.0 + 0.5)
        SCH_CLAMP = -1400.0   # raw-score clamp: keeps bits >= 1 (no int8 wrap)

        def attn_block(b, interleave, dve_exp=False):
            """interleave: list of callables, one consumed per (hp, grp) step
            (run BEFORE that step's scores). dve_exp: run the softmax exp of
            every 4th step on DVE via the Schraudolph bit trick instead of
            ACT (used in the early, ACT-bound blocks)."""
            qs = slice(b * QB, (b + 1) * QB)
            attnT = ptile.tile([P, HT, QB], FP8, name="attnT", tag="attnT")
            attn_blk[b] = attnT
            step = 0
            for hp in range(HT):
                oab = pp.tile([P, 512], F32, name="oab", tag="PB0")
                for grp in range(4):
                    if step < len(interleave):
                        interleave[step]()
                    sa = pp.tile([P, 1024], F32, name="sa", tag="PB45")
                    sb = pp.tile([P, 1024], F32, name="sb", tag="PB67")
                    for i in range(4):
                        kt = grp * 4 + i
                        nc.tensor.matmul(sa[:, i * QB:(i + 1) * QB],
                                         lhsT=KT[hp][0:HD, kt * P:(kt + 1) * P],
                                         rhs=QT[hp][0:HD, qs], start=True, stop=True)
                    for i in range(4):
                        kt = grp * 4 + i
                        nc.tensor.matmul(sb[:, i * QB:(i + 1) * QB],
                                         lhsT=KT[hp][HD:P, kt * P:(kt + 1) * P],
                                         rhs=QT[hp][HD:P, qs], start=True, stop=True)
                    if dve_exp and step % 4 == 3:
                        pa = ptile.tile([P, 4, QB], mybir.dt.int8, name="pa8", tag="pa")
                        pb = ptile.tile([P, 4, QB], mybir.dt.int8, name="pb8", tag="pb")
                        for s_ps, p8 in ((sa, pa), (sb, pb)):
                            scb = ptile.tile([P, 1024], BF16, name="scb", tag="sch", bufs=1)
                            nc.vector.tensor_scalar(out=scb, in0=s_ps, scalar1=SCH_CLAMP,
                                                    scalar2=None, op0=mybir.AluOpType.max)
                            nc.vector.tensor_scalar(out=p8.rearrange("p a b -> p (a b)"),
                                                    in0=scb, scalar1=SCH_A, scalar2=SCH_B,
                                                    op0=mybir.AluOpType.mult,
                                                    op1=mybir.AluOpType.add)
                        pa = pa.bitcast(FP8)
                        pb = pb.bitcast(FP8)
                    else:
                        pa = ptile.tile([P, 4, QB], FP8, name="pa", tag="pa")
                        pb = ptile.tile([P, 4, QB], FP8, name="pb", tag="pb")
                        nc.scalar.activation(out=pa, in_=sa.rearrange("p (a b) -> p a b", a=4),
                                             func=mybir.ActivationFunctionType.Exp,
                                             bias=expb, scale=EXP_SCALE)
                        nc.scalar.activation(out=pb, in_=sb.rearrange("p (a b) -> p a b", a=4),
                                             func=mybir.ActivationFunctionType.Exp,
                                             bias=expb, scale=EXP_SCALE)
                    for pr in range(2):
                        kp = grp * 2 + pr
                        first = (grp == 0 and pr == 0)
                        last = (grp == 3 and pr == 1)
                        # oa/ob share one PSUM bank and ONE accumulation
                        # group: start only on the very first matmul (zeroes
                        # the whole 2KB region), stop only on the very last.
                        nc.tensor.matmul(oab[0:HD2, 0:QB], lhsT=VS[kp][:, 2 * hp, :, :],
                                         rhs=pa[:, 2 * pr:2 * pr + 2, :],
                                         start=first, stop=False, perf_mode=DR,
                                         skip_group_check=True)
                        nc.tensor.matmul(oab[0:HD2, QB:2 * QB], lhsT=VS[kp][:, 2 * hp + 1, :, :],
                                         rhs=pb[:, 2 * pr:2 * pr + 2, :],
                                         start=False, stop=last, perf_mode=DR,
                                         skip_group_check=True)
                    step += 1
                r = rtile.tile([1, 512], F32, name="r_recip", tag="r_recip")
                nc.vector.reciprocal(out=r, in_=oab[HD:HD1, :])
                rb = rtile.tile([HD, 512], F32, name="r_bcast", tag="r_bcast", bufs=1)
                nc.gpsimd.partition_broadcast(rb, r)
                nc.vector.tensor_tensor(out=attnT[0:HD, hp, :], in0=oab[0:HD, 0:QB],
                                        in1=rb[:, 0:QB], op=mybir.AluOpType.mult)
                nc.vector.tensor_tensor(out=attnT[HD:P, hp, :], in0=oab[0:HD, QB:2 * QB],
                                        in1=rb[:, QB:2 * QB], op=mybir.AluOpType.mult)
            for fn in interleave[step:]:
                fn()

        def outproj_ln2(b):
            for tt2 in range(2 * b, 2 * b + 2):
                xres = xload.tile([P, H], F32, name="xres", tag="xres", bufs=2)
                nc.sync.dma_start(out=xres, in_=xin[tt2 * P:(tt2 + 1) * P, :])
                nc.vector.tensor_tensor(out=xres, in0=xres, in1=bo_bc, op=mybir.AluOpType.add)
                x1t = xres
                for c in range(2):
                    ps = ps512(tags=("PB1", "PB2", "PB3"))
                    for t2 in range(4):
                        nc.tensor.matmul(ps,
                                         lhsT=attn_blk[b][:, 2 * t2:2 * t2 + 2,
                                                          (tt2 - 2 * b) * P:(tt2 - 2 * b + 1) * P],
                                         rhs=wo_a[:, 2 * t2:2 * t2 + 2, c * 512:(c + 1) * 512],
                                         start=(t2 == 0), stop=(t2 == 3), perf_mode=DR)
                    nc.vector.scalar_tensor_tensor(out=x1t[:, c * 512:(c + 1) * 512],
                                                   in0=ps, scalar=OP_DESCALE,
                                                   in1=xres[:, c * 512:(c + 1) * 512],
                                                   op0=mybir.AluOpType.mult,
                                                   op1=mybir.AluOpType.add)
                nc.sync.dma_start(out=x1_dram[tt2 * P:(tt2 + 1) * P, :], in_=x1t)
                nb2 = nbp.tile([P, H], BF16, name="nb2", tag="nb")
                _ln_tile(nc, stat, x1t, nb2, apply_on_act=False)
                nc.sync.dma_start_transpose(out=n2T[:, :, tt2 * P:(tt2 + 1) * P], in_=nb2)

        # MLP over 256-token chunks cb (= attention block granularity)
        def mlp1_steps(cb, tags=("PB1",)):
            """Returns list of 8 closures; each runs 4 ft tiles of MLP1."""
            ts_ = slice(cb * QB, (cb + 1) * QB)
            steps = []
            for g in range(8):
                def run(g=g):
                    w1g = w1p.tile([P, HT, 512], BF16, name="w1g", tag="w1g")
                    nc.gpsimd.dma_start(out=w1g, in_=wm1T[:, g * 512:(g + 1) * 512]
                                        .rearrange("(a p) c -> p a c", p=P))
                    for f2 in range(4):
                        ft = g * 4 + f2
                        ps = ps512(tags=tags)
                        psh = ps[:, 0:QB]
                        for ht in range(HT):
                            nc.tensor.matmul(psh, lhsT=w1g[:, ht, f2 * P:(f2 + 1) * P],
                                             rhs=n2T[:, ht, ts_],
                                             start=(ht == 0), stop=(ht == HT - 1))
                        nc.vector.tensor_scalar(out=h1[cb][:, ft, :], in0=psh,
                                                scalar1=bm1d_t[:, ft:ft + 1], scalar2=None,
                                                op0=mybir.AluOpType.add)
                steps.append(run)
            return steps

        def gelu_batch(cb):
            nc.scalar.activation(out=h1[cb], in_=h1[cb],
                                 func=mybir.ActivationFunctionType.Gelu,
                                 bias=zerob, scale=1.0)

        def mlp2_steps(cb):
            """Returns list of 8 closures: (c, g) quarters accumulating into
            PB2/PB3 [tl0, tl1]; drains emitted in the last closure per c."""
            steps = []
            for c in range(2):
                pstl = [None, None]
                for g in range(8):
                    def run(c=c, g=g, pstl=pstl):
                        if g == 0:
                            pstl[0] = pp.tile([P, 512], F32, name="m2a", tag="PB2")
                            pstl[1] = pp.tile([P, 512], F32, name="m2b", tag="PB3")
                        w2g = w2p.tile([P, 4, 512], BF16, name="w2g", tag="w2g")
                        nc.gpsimd.dma_start(
                            out=w2g,
                            in_=wm2T[g * 512:(g + 1) * 512, c * 512:(c + 1) * 512]
                            .rearrange("(a p) c2 -> p a c2", p=P))
                        for f2 in range(4):
                            ft = g * 4 + f2
                            for tl in range(2):
                                nc.tensor.matmul(pstl[tl],
                                                 lhsT=h1[cb][:, ft, tl * P:(tl + 1) * P],
                                                 rhs=w2g[:, f2, :],
                                                 start=(ft == 0), stop=(ft == FT - 1))
                        if g == 7:
                            for tl in range(2):
                                tt2 = cb * 2 + tl
                                sl = slice(c * 512, (c + 1) * 512)
                                x1r = oload.tile([P, 512], F32, name="x1r", tag="x1r")
                                nc.sync.dma_start(out=x1r, in_=x1_dram[tt2 * P:(tt2 + 1) * P, sl])
                                ot = oload.tile([P, 512], F32, name="out_t", tag="out_t")
                                nc.vector.tensor_tensor(out=ot, in0=pstl[tl], in1=bm2_bc[:, sl],
                                                        op=mybir.AluOpType.add)
                                nc.vector.tensor_tensor(out=ot, in0=ot, in1=x1r,
                                                        op=mybir.AluOpType.add)
                                nc.sync.dma_start(out=out[tt2 * P:(tt2 + 1) * P, sl], in_=ot)
                    steps.append(run)
            return steps

        # ---- schedule ----
        # b0 carries the deferred phase-1 tail; MLP chunk cb rides in b >= cb+1.
        attn_block(0, [
            lambda: (phase1_tt(8), phase1_tt(9)),
            lambda: (phase1_tt(10), phase1_tt(11)),
            lambda: (phase1_kq(2), phase1_tt(12), phase1_tt(13)),
            lambda: (phase1_tt(14), phase1_tt(15), phase1_kq(3)),
        ], dve_exp=False)
        outproj_ln2(0)
        noop = lambda: None
        attn_block(1, [noop] * 4 + mlp1_steps(0), dve_exp=False)
        outproj_ln2(1)
        gelu_batch(0)
        attn_block(2, mlp2_steps(0) + mlp1_steps(1))
        outproj_ln2(2)
        gelu_batch(1)
        attn_block(3, mlp2_steps(1) + mlp1_steps(2))
        outproj_ln2(3)
        gelu_batch(2)
        m13 = mlp1_steps(3, tags=("PB1", "PB2", "PB3"))
        for fn in m13[:4]:
            fn()
        nc.scalar.activation(out=h1[3][:, 0:FT // 2, :], in_=h1[3][:, 0:FT // 2, :],
                             func=mybir.ActivationFunctionType.Gelu,
                             bias=zerob, scale=1.0)
        for fn in m13[4:]:
            fn()
        nc.scalar.activation(out=h1[3][:, FT // 2:FT, :], in_=h1[3][:, FT // 2:FT, :],
                             func=mybir.ActivationFunctionType.Gelu,
                             bias=zerob, scale=1.0)
        # chunks 2+3 MLP2 in one pass: 4 open accumulators, wm2 streamed once
        for c in range(2):
            m2x = pp.tile([P, 1024], F32, name="m2x", tag="PB45")
            m2y = pp.tile([P, 1024], F32, name="m2y", tag="PB67")
            accs = [m2x[:, 0:512], m2x[:, 512:1024], m2y[:, 0:512], m2y[:, 512:1024]]
            for g in range(8):
                w2g = w2p.tile([P, 4, 512], BF16, name="w2g", tag="w2g")
                nc.gpsimd.dma_start(
                    out=w2g,
                    in_=wm2T[g * 512:(g + 1) * 512, c * 512:(c + 1) * 512]
                    .rearrange("(a p) c2 -> p a c2", p=P))
                for f2 in range(4):
                    ft = g * 4 + f2
                    for tl in range(4):
                        nc.tensor.matmul(accs[tl],
                                         lhsT=h1[2 + tl // 2][:, ft, (tl % 2) * P:(tl % 2 + 1) * P],
                                         rhs=w2g[:, f2, :],
                                         start=(ft == 0), stop=(ft == FT - 1))
            for tl in range(4):
                tt2 = 4 + tl
                sl = slice(c * 512, (c + 1) * 512)
                x1r = oload.tile([P, 512], F32, name="x1r", tag="x1r")
                nc.sync.dma_start(out=x1r, in_=x1_dram[tt2 * P:(tt2 + 1) * P, sl])
                ot = oload.tile([P, 512], F32, name="out_t", tag="out_t")
                nc.vector.tensor_tensor(out=ot, in0=accs[tl], in1=bm2_bc[:, sl],
                                        op=mybir.AluOpType.add)
                nc.vector.tensor_tensor(out=ot, in0=ot, in1=x1r,
                                        op=mybir.AluOpType.add)
                nc.sync.dma_start(out=out[tt2 * P:(tt2 + 1) * P, sl], in_=ot)

    nc.finalize()
    return nc


def _prep_host_inputs(x, Wq, bq, Wk, bk, Wv, bv, Wo, bo,
                      g1, b1, g2, b2, Wm1, bm1, Wm2, bm2):
    """Fold LN affine params into weights, transpose, scale, cast."""
    f32 = np.float32
    g1 = np.asarray(g1, f32); b1 = np.asarray(b1, f32)
    g2 = np.asarray(g2, f32); b2 = np.asarray(b2, f32)
    Wq = np.asarray(Wq, f32); Wk = np.asarray(Wk, f32); Wv = np.asarray(Wv, f32)
    Wo = np.asarray(Wo, f32); Wm1 = np.asarray(Wm1, f32); Wm2 = np.asarray(Wm2, f32)

    bf = ml_dtypes.bfloat16
    f8 = ml_dtypes.float8_e4m3
    wq8 = np.ascontiguousarray(Wq.T * g1[:, None] * 8.0).astype(f8)   # [h, d]
    wk8 = np.ascontiguousarray(Wk.T * g1[:, None] * 8.0).astype(f8)
    wv8 = np.ascontiguousarray(Wv.T * g1[:, None] * 16.0).astype(f8)
    wo8 = np.ascontiguousarray(Wo.T * 8.0).astype(f8)                 # [d, ho]
    wm1T = np.ascontiguousarray(Wm1.T * g2[:, None]).astype(bf)       # [h, f]
    wm2T = np.ascontiguousarray(Wm2.T).astype(bf)                     # [f, ho]

    bq_f = ((b1 @ Wq.T + np.asarray(bq, f32)) * 8.0).astype(f32)
    bk_f = ((b1 @ Wk.T + np.asarray(bk, f32)) * 8.0).astype(f32)
    bv_f = ((b1 @ Wv.T + np.asarray(bv, f32)) * 16.0).astype(f32)
    bm1_f = (b2 @ Wm1.T + np.asarray(bm1, f32)).astype(f32)

    shared = {
        "wq8": wq8, "wk8": wk8, "wv8": wv8, "wo8": wo8,
        "wm1T": wm1T, "wm2T": wm2T,
        "bqd": bq_f.reshape(HT, P), "bkd": bk_f.reshape(HT, P),
        "bvv": bv_f.reshape(1, H).astype(bf),
        "bov": np.asarray(bo, f32).reshape(1, H).astype(bf),
        "bm1d": bm1_f.reshape(FT, P),
        "bm2v": np.asarray(bm2, f32).reshape(1, H).astype(bf),
    }
    x = np.asarray(x, f32)
    in_maps = []
    for c in range(8):
        b_i, q_i = c // 2, c % 2
        xb = x[b_i]
        xin = np.ascontiguousarray(
            np.concatenate([xb[q_i * QT_N:], xb[:q_i * QT_N]], axis=0))
        in_maps.append({"xin": xin, **shared})
    return in_maps


def run_device(in_maps, core_ids=None, **kwargs):
    if "nc" not in _CACHED:
        _CACHED["nc"] = build_core_kernel()
    nc = _CACHED["nc"]
    if core_ids is None:
        core_ids = list(range(len(in_maps)))
    return run_bass_kernel_spmd(nc, in_maps, core_ids=core_ids, **kwargs)


def kernel(x, attention_mask, Wq, bq, Wk, bk, Wv, bv, Wo, bo,
           g1, b1, g2, b2, Wm1, bm1, Wm2, bm2):
    del attention_mask  # all-ones by construction of the problem inputs
    in_maps = _prep_host_inputs(x, Wq, bq, Wk, bk, Wv, bv, Wo, bo,
                                g1, b1, g2, b2, Wm1, bm1, Wm2, bm2)
    res = run_device(in_maps)
    outf = np.empty((B, S, H), np.float32)
    for c in range(8):
        b_i, q_i = c // 2, c % 2
        outf[b_i, q_i * QT_N:(q_i + 1) * QT_N] = res.results[c]["out"]
    return outf
